# revision 50
# baseline (speedup 1.0000x reference)
"""PASA group-softmax downsample kernel for 8 Trainium2 NeuronCores. V3.

Pipeline per core (1 sample-half, 32 out rows, 4 row blocks):
  DMA xpad block (fp32r) ->
  [PE]  strided 3x3 conv as 18 accumulating fp32r matmuls -> logits PSUM [72,512]
  [ACT] exp(logits + bias) -> sexp bf16
  [PE]  ones-matmul denominator; [ACT] copy row; [DVE] reciprocal (DMA reshape)
  [PE]  K=1 broadcast of recip to 72 rows (rps72 PSUM)
  [DVE] sig = sexp * rps72  (pre-normalized softmax weights, bf16)
  per tap t, chunk ch:
    [PE]  selection matmul broadcasts sig rows -> sbp PSUM [128,512]
    either [DVE] pk = sbp * x  (stt from PSUM)
    or     [ACT] sbc = bf16(sbp); [Pool] pk = sbc * x
  sum of 9 pk:
    [PE]  identity-matmul accumulation of `pe_leaves` pks into Sacc PSUM
    [DVE/Pool] small add tree over the remaining pks -> T
    [DVE] out = Sacc + T (stt, writes fp32) -> DMA out.

Whole body sits in a For_i(0, nrep) hardware loop for self-timing.
"""

import numpy as np
from contextlib import ExitStack

import ml_dtypes

import concourse.bass as bass
import concourse.bacc as bacc
import concourse.tile as tile
import concourse.mybir as mybir
import concourse.bass_utils as bass_utils
from concourse.ap import AP

F32 = mybir.dt.float32
F32R = mybir.dt.float32r
BF16 = mybir.dt.bfloat16
ALU = mybir.AluOpType
ACTF = mybir.ActivationFunctionType

N_CORES = 8
N, C, H, W = 4, 256, 128, 128
K = 3
GROUP = 8
CO = GROUP * K * K          # 72 conv output channels
OH, OW = H // 2, W // 2     # 64 x 64 output spatial
HALF = OH // 2              # 32 output rows per core
BN_EPS = 1e-5

NBLK = 4                    # row blocks per core
BROWS = HALF // NBLK        # 8 output rows per block
BPIX = BROWS * OW           # 512 pixels per block
XROWS = 2 * BROWS + 1       # 17 xp rows per block
XW = W + 1                  # 129 padded cols (left reflect only)

# build sections (label, first-instruction-id), for timeline analysis
_SECTIONS = []
_SEC_NC = None


def _sec(label):
    if _SEC_NC is not None:
        _SECTIONS.append((label, int(_SEC_NC.next_id())))


# ---- tuning knobs ----
CFG = dict(
    # per-chunk tap sets multiplied via ACT-copy + Pool (rest: DVE stt)
    pool_taps={0: (1, 3, 5), 1: (1, 3, 5)},
    # taps whose pk is accumulated on PE via identity matmuls
    pe_leaves=(0, 1, 2, 3, 5),
    # engine for each tree merge (per unit, cycled by unit index):
    # list of per-merge engines for the tree over the non-PE leaves
    merge_engines=(("vector", "vector", "gpsimd"),
                   ("vector", "gpsimd", "vector")),
    # use scalar_tensor_tensor on gpsimd (REJECTED by walrus codegen —
    # TensorScalarPtr is not a Pool-engine opcode; keep False)
    pool_stt=False,
    # scheduler priority boosts (None=off, int=priority offset, 0=front)
    hp_chain=None,
    hp_xdma=None,
    sigb_bufs=3,
    sacc_bufs=2,
    pk_bufs=14,
    acc_bufs=4,
    sbc_bufs=4,
    conv_bufs=1,
    xpad_bufs=2,
)


def build_kernel(nc, cfg=CFG, timing_loop=True, unroll=1):
    global _SEC_NC
    _SEC_NC = nc
    _SECTIONS.clear()
    xh = nc.dram_tensor("xh", [C, 2 * HALF + 1, XW], F32R, kind="ExternalInput").ap()
    wconv = nc.dram_tensor("wconv", [128, 36 * CO], F32R, kind="ExternalInput").ap()
    sel = nc.dram_tensor("sel", [CO, 36 * 128], mybir.dt.float8e4, kind="ExternalInput").ap()
    bias = nc.dram_tensor("bias", [CO, 1], F32, kind="ExternalInput").ap()
    ones72 = nc.dram_tensor("ones72", [CO, 1], BF16, kind="ExternalInput").ap()
    o1x128 = nc.dram_tensor("o1x128", [1, 128], F32R, kind="ExternalInput").ap()
    ident = nc.dram_tensor("ident", [128, 128], BF16, kind="ExternalInput").ap()
    nrep = nc.dram_tensor("nrep", [1, 1], mybir.dt.int32, kind="ExternalInput").ap()
    oh = nc.dram_tensor("oh", [C, HALF, OW], F32, kind="ExternalOutput").ap()

    pool_taps = {ch: set(v) for ch, v in cfg["pool_taps"].items()}
    pe_leaves = tuple(cfg["pe_leaves"])
    merge_engines = cfg["merge_engines"]
    tree_taps = [t for t in range(9) if t not in pe_leaves]

    with tile.TileContext(nc) as tc, ExitStack() as ctx:
        cpool = ctx.enter_context(tc.tile_pool(name="consts", bufs=1))
        xpool = ctx.enter_context(tc.tile_pool(name="xpad", bufs=cfg["xpad_bufs"]))
        spool = ctx.enter_context(tc.tile_pool(name="small", bufs=3))
        sbcool = ctx.enter_context(tc.tile_pool(name="sbc", bufs=cfg["sbc_bufs"]))
        pkpool = ctx.enter_context(tc.tile_pool(name="pk", bufs=cfg["pk_bufs"]))
        accpool = ctx.enter_context(tc.tile_pool(name="acc", bufs=cfg["acc_bufs"]))
        outpool = ctx.enter_context(tc.tile_pool(name="outs", bufs=2))
        convp = ctx.enter_context(tc.tile_pool(name="convp", bufs=cfg["conv_bufs"], space="PSUM"))
        sgbp = ctx.enter_context(tc.tile_pool(name="sgbp", bufs=cfg["sigb_bufs"], space="PSUM"))
        denp = ctx.enter_context(tc.tile_pool(name="denp", bufs=1, space="PSUM"))
        rpsp = ctx.enter_context(tc.tile_pool(name="rpsp", bufs=1, space="PSUM"))
        saccp = ctx.enter_context(tc.tile_pool(name="saccp", bufs=cfg["sacc_bufs"], space="PSUM"))

        wsb = cpool.tile([128, 36 * CO], F32R, tag="wsb")
        SPLIT = 4 * CO
        nc.sync.dma_start(wsb[:, 0:SPLIT], wconv[:, 0:SPLIT])
        nc.sync.dma_start(wsb[:, SPLIT:], wconv[:, SPLIT:])
        selsb = cpool.tile([CO, 36 * 128], mybir.dt.float8e4, tag="selsb")
        nc.sync.dma_start(selsb[:], sel)
        bsb = cpool.tile([CO, 1], F32, tag="bsb")
        nc.sync.dma_start(bsb[:], bias)
        o72sb = cpool.tile([CO, 1], BF16, tag="o72sb")
        nc.sync.dma_start(o72sb[:], ones72)
        o1x128sb = cpool.tile([1, 128], F32R, tag="o1x128sb")
        nc.sync.dma_start(o1x128sb[:], o1x128)
        idsb = cpool.tile([128, 128], BF16, tag="idsb")
        nc.sync.dma_start(idsb[:], ident)
        rsb_t = cpool.tile([1, 1], mybir.dt.int32, tag="rsb_t")
        nc.sync.dma_start(rsb_t[:], nrep)
        if timing_loop:
            with tc.tile_critical():
                nrep_v = nc.values_load(rsb_t[:], min_val=1, max_val=1 << 20,
                                        skip_runtime_bounds_check=True)
            loop_cm = tc.For_i(0, nrep_v, 1, hint_engines=(
                mybir.EngineType.PE, mybir.EngineType.DVE,
                mybir.EngineType.Activation, mybir.EngineType.Pool,
                mybir.EngineType.SP))
        else:
            import contextlib
            loop_cm = contextlib.nullcontext()
        with loop_cm:
            for b in range(NBLK * unroll):
                b = b % NBLK
                xts = []
                _sec(f"xdma b{b}")
                import contextlib

                def prio(key):
                    v = cfg.get(key)
                    if v is None:
                        return contextlib.nullcontext()
                    return tc.high_priority(v if v > 0 else None)

                with prio("hp_xdma"):
                    for ch in range(2):
                        xt = xpool.tile([128, XROWS * XW], F32R, tag=f"x{ch}")
                        src = xh[128 * ch:128 * (ch + 1), 16 * b:16 * b + XROWS, :]
                        nc.sync.dma_start(
                            xt[:].rearrange("p (h w) -> p h w", w=XW), src)
                        xts.append(xt)

                def xview(t, ch):
                    ki, kj = divmod(t, 3)
                    v = xts[ch][:].rearrange("p (h w) -> p h w", w=XW)
                    return v[:, ki:ki + 2 * BROWS - 1:2, kj:kj + 2 * OW - 1:2]

                # ---- strided conv: 18 accumulating fp32r matmuls ----
                # row 72 of the tile doubles as the softmax-denominator row
                _sec(f"conv b{b}")
                cps_t = convp.tile([CO, BPIX], F32, tag="cps")
                cps = cps_t[:]
                for t in range(9):
                    for ch in range(2):
                        wv = wsb[:, (t * 2 + ch) * CO:(t * 2 + ch + 1) * CO]
                        nc.tensor.matmul(
                            cps.rearrange("p (h w) -> p h w", w=OW),
                            wv, xview(t, ch),
                            start=(t == 0 and ch == 0),
                            stop=(t == 8 and ch == 1))

                # ---- BN bias + exp (-> bf16) ----
                _sec(f"exp b{b}")
                with prio("hp_chain"):
                    sexp = spool.tile([CO, BPIX], BF16, tag="sexp")
                    nc.scalar.activation(sexp[:], cps[:], ACTF.Exp,
                                         bias=bsb[:], scale=1.0)

                    # ---- softmax denominator + reciprocal ----
                    _sec(f"den b{b}")
                    dps_t = denp.tile([1, BPIX], F32, tag="dps")
                    dps = dps_t[:]
                    nc.tensor.matmul(dps[:], o72sb[:], sexp[:],
                                     start=True, stop=True)
                    rrow = spool.tile([1, BPIX], F32, tag="rrow")
                    with nc.allow_low_precision(reason="18-bit recip, tol 2e-2"):
                        nc.vector.reciprocal_approx_fast(rrow[:], dps[:])
                    rrowr = spool.tile([1, BPIX], F32R, tag="rrowr")
                    nc.scalar.copy(rrowr[:], rrow[:])

                    # ---- broadcast recip to 128 rows, copy to SBUF bf16 ----
                    _sec(f"norm b{b}")
                    rps = rpsp.tile([128, BPIX], F32, tag="rps")
                    nc.tensor.matmul(rps[:], o1x128sb[:], rrowr[:],
                                     start=True, stop=True)
                    rB = spool.tile([128, BPIX], BF16, tag="rB")
                    nc.scalar.copy(rB[:], rps[:])
                sig = sexp

                # ---- per chunk: broadcast + weighted tap sum ----
                for ch in range(2):
                    unit = 2 * b + ch
                    pks = [None] * 9
                    for t in range(9):
                        _sec(f"tap{t} u{unit}")
                        sbp = sgbp.tile([128, BPIX], F32, tag="sbp")
                        sv = selsb[:, (t * 2 + ch) * 128:(t * 2 + ch + 1) * 128]
                        nc.tensor.matmul(sbp[:], sv, sig[:],
                                         start=True, stop=True)
                        pk = pkpool.tile([128, BPIX], BF16, tag="pk")
                        if t in pool_taps[ch]:
                            sbc = sbcool.tile([128, BPIX], BF16, tag="sbc")
                            nc.scalar.copy(sbc[:], sbp[:])
                            if cfg.get("pool_stt"):
                                nc.gpsimd.scalar_tensor_tensor(
                                    pk[:].rearrange("p (h w) -> p h w", w=OW),
                                    sbc[:].rearrange("p (h w) -> p h w", w=OW),
                                    1.0, xview(t, ch).bitcast(F32),
                                    ALU.bypass, ALU.mult)
                            else:
                                nc.gpsimd.tensor_tensor(
                                    pk[:].rearrange("p (h w) -> p h w", w=OW),
                                    sbc[:].rearrange("p (h w) -> p h w", w=OW),
                                    xview(t, ch).bitcast(F32), ALU.mult)
                        else:
                            nc.vector.scalar_tensor_tensor(
                                pk[:].rearrange("p (h w) -> p h w", w=OW),
                                sbp[:].rearrange("p (h w) -> p h w", w=OW),
                                1.0, xview(t, ch).bitcast(F32),
                                ALU.bypass, ALU.mult)
                        pks[t] = pk

                    # PE identity-matmul accumulation of pe_leaves
                    _sec(f"sacc u{unit}")
                    sacc = saccp.tile([128, BPIX], F32, tag="sacc")
                    for i, t in enumerate(pe_leaves):
                        nc.tensor.matmul(sacc[:], idsb[:], pks[t][:],
                                         start=(i == 0), stop=False)

                    # small add tree over the rest, injected into sacc by PE
                    _sec(f"tree u{unit}")
                    lvl = [pks[t] for t in tree_taps]
                    mi = 0
                    mengs = merge_engines[unit % len(merge_engines)]
                    while len(lvl) > 1:
                        nxt = []
                        for i in range(0, len(lvl) - 1, 2):
                            s = accpool.tile([128, BPIX], BF16, tag="acc")
                            eng = getattr(nc, mengs[mi % len(mengs)])
                            mi += 1
                            if eng is nc.gpsimd and cfg.get("pool_stt"):
                                eng.scalar_tensor_tensor(
                                    s[:], lvl[i][:], 1.0, lvl[i + 1][:],
                                    ALU.bypass, ALU.add)
                            else:
                                eng.tensor_add(s[:], lvl[i][:], lvl[i + 1][:])
                            nxt.append(s)
                        if len(lvl) % 2:
                            nxt.append(lvl[-1])
                        lvl = nxt
                    nc.tensor.matmul(sacc[:], idsb[:], lvl[0][:],
                                     start=False, stop=True)

                    # normalize from PSUM and write fp32 out
                    _sec(f"final u{unit}")
                    outsb = outpool.tile([128, BPIX], F32, tag="outsb")
                    nc.vector.scalar_tensor_tensor(
                        outsb[:], sacc[:], 1.0, rB[:], ALU.bypass, ALU.mult)
                    nc.sync.dma_start(
                        oh[128 * ch:128 * (ch + 1), BROWS * b:BROWS * (b + 1), :],
                        outsb[:].rearrange("p (h w) -> p h w", w=OW))
    nc.compile()
    return nc


def prepare_const_inputs(conv_w, bn_gamma, bn_beta, bn_mean, bn_var):
    inv = 1.0 / np.sqrt(bn_var.astype(np.float64) + BN_EPS)
    scale = (bn_gamma.astype(np.float64) * inv)
    wp = conv_w.astype(np.float64) * scale[:, None, None, None]
    bias = (bn_beta.astype(np.float64)
            - bn_mean.astype(np.float64) * scale).astype(np.float32)

    wconv = np.zeros((128, 36, CO), np.float32)
    selm = np.zeros((CO, 36, 128), np.float32)
    for t in range(9):
        ki, kj = divmod(t, 3)
        for ch in range(2):
            blk = t * 2 + ch
            wconv[:, blk, :] = wp[:, 128 * ch:128 * (ch + 1), ki, kj].T
            for g4 in range(4):
                r = (ch * 4 + g4) * 9 + t
                selm[r, blk, 32 * g4:32 * (g4 + 1)] = 1.0
    return {
        "wconv": np.ascontiguousarray(wconv.reshape(128, 36 * CO)),
        "sel": np.ascontiguousarray(
            selm.reshape(CO, 36 * 128)).astype(ml_dtypes.float8_e4m3),
        "bias": bias.reshape(CO, 1),
        "ones72": np.ones((CO, 1), ml_dtypes.bfloat16),
        "o1x128": np.ones((1, 128), np.float32),
        "ident": np.eye(128, dtype=ml_dtypes.bfloat16),
    }


def prepare_x_core(x, core):
    n, half = divmod(core, 2)
    if half == 0:
        rows = np.concatenate([x[n, :, 1:2, :], x[n, :, 0:2 * HALF, :]], axis=1)
    else:
        rows = x[n, :, 2 * HALF - 1:H, :]
    padded = np.concatenate([rows[:, :, 1:2], rows], axis=2)
    return np.ascontiguousarray(padded.astype(np.float32))


_CACHE = {}


def _get_nc(key="v5", cfg=None, timing_loop=True):
    if key not in _CACHE:
        nc = bacc.Bacc("TRN2", target_bir_lowering=False, debug=False,
                       num_devices=N_CORES)
        _CACHE[key] = build_kernel(nc, cfg=cfg or CFG, timing_loop=timing_loop)
    return _CACHE[key]


def run_on_cores(inputs, reps=1):
    nc = _get_nc()
    consts = prepare_const_inputs(
        inputs["conv_w"], inputs["bn_gamma"], inputs["bn_beta"],
        inputs["bn_mean"], inputs["bn_var"])
    consts["nrep"] = np.array([[reps]], np.int32)
    x = np.asarray(inputs["x"])
    in_maps = []
    for core in range(N_CORES):
        m = dict(consts)
        m["xh"] = prepare_x_core(x, core)
        in_maps.append(m)
    res = bass_utils.run_bass_kernel_spmd(nc, in_maps, core_ids=list(range(N_CORES)))
    out = np.empty((N, C, OH, OW), np.float32)
    for core in range(N_CORES):
        n, half = divmod(core, 2)
        out[n, :, HALF * half:HALF * (half + 1), :] = res.results[core]["oh"]
    return out


def kernel(**inputs):
    return run_on_cores(inputs, reps=1)


# revision 51
# speedup vs baseline: 12.0713x; 12.0713x over previous
"""PASA group-softmax downsample kernel for 8 Trainium2 NeuronCores. V3.

Pipeline per core (1 sample-half, 32 out rows, 4 row blocks):
  DMA xpad block (fp32r) ->
  [PE]  strided 3x3 conv as 18 accumulating fp32r matmuls -> logits PSUM [72,512]
  [ACT] exp(logits + bias) -> sexp bf16
  [PE]  ones-matmul denominator; [ACT] copy row; [DVE] reciprocal (DMA reshape)
  [PE]  K=1 broadcast of recip to 72 rows (rps72 PSUM)
  [DVE] sig = sexp * rps72  (pre-normalized softmax weights, bf16)
  per tap t, chunk ch:
    [PE]  selection matmul broadcasts sig rows -> sbp PSUM [128,512]
    either [DVE] pk = sbp * x  (stt from PSUM)
    or     [ACT] sbc = bf16(sbp); [Pool] pk = sbc * x
  sum of 9 pk:
    [PE]  identity-matmul accumulation of `pe_leaves` pks into Sacc PSUM
    [DVE/Pool] small add tree over the remaining pks -> T
    [DVE] out = Sacc + T (stt, writes fp32) -> DMA out.

Whole body sits in a For_i(0, nrep) hardware loop for self-timing.
"""

import numpy as np
from contextlib import ExitStack

import ml_dtypes

import concourse.bass as bass
import concourse.bacc as bacc
import concourse.tile as tile
import concourse.mybir as mybir
import concourse.bass_utils as bass_utils
from concourse.ap import AP

F32 = mybir.dt.float32
F32R = mybir.dt.float32r
BF16 = mybir.dt.bfloat16
ALU = mybir.AluOpType
ACTF = mybir.ActivationFunctionType

N_CORES = 8
N, C, H, W = 4, 256, 128, 128
K = 3
GROUP = 8
CO = GROUP * K * K          # 72 conv output channels
OH, OW = H // 2, W // 2     # 64 x 64 output spatial
HALF = OH // 2              # 32 output rows per core
BN_EPS = 1e-5

NBLK = 4                    # row blocks per core
BROWS = HALF // NBLK        # 8 output rows per block
BPIX = BROWS * OW           # 512 pixels per block
XROWS = 2 * BROWS + 1       # 17 xp rows per block
XW = W + 1                  # 129 padded cols (left reflect only)

# build sections (label, first-instruction-id), for timeline analysis
_SECTIONS = []
_SEC_NC = None


def _sec(label):
    if _SEC_NC is not None:
        _SECTIONS.append((label, int(_SEC_NC.next_id())))


# ---- tuning knobs ----
CFG = dict(
    # per-chunk tap sets multiplied via ACT-copy + Pool (rest: DVE stt)
    pool_taps={0: (1, 3, 5), 1: (1, 3, 5)},
    # taps whose pk is accumulated on PE via identity matmuls
    pe_leaves=(0, 1, 2, 3, 4),
    # engine for each tree merge (per unit, cycled by unit index):
    # list of per-merge engines for the tree over the non-PE leaves
    merge_engines=(("vector", "vector", "gpsimd"),
                   ("vector", "gpsimd", "vector")),
    # use scalar_tensor_tensor on gpsimd (REJECTED by walrus codegen —
    # TensorScalarPtr is not a Pool-engine opcode; keep False)
    pool_stt=False,
    # scheduler priority boosts (None=off, int=priority offset, 0=front)
    hp_chain=None,
    hp_xdma=None,
    sigb_bufs=3,
    sacc_bufs=2,
    pk_bufs=14,
    acc_bufs=4,
    sbc_bufs=4,
    conv_bufs=1,
    xpad_bufs=2,
)


def build_kernel(nc, cfg=CFG, timing_loop=True, unroll=1):
    global _SEC_NC
    _SEC_NC = nc
    _SECTIONS.clear()
    xh = nc.dram_tensor("xh", [C, 2 * HALF + 1, XW], F32R, kind="ExternalInput").ap()
    wconv = nc.dram_tensor("wconv", [128, 36 * CO], F32R, kind="ExternalInput").ap()
    sel = nc.dram_tensor("sel", [CO, 36 * 128], mybir.dt.float8e4, kind="ExternalInput").ap()
    bias = nc.dram_tensor("bias", [CO, 1], F32, kind="ExternalInput").ap()
    ones72 = nc.dram_tensor("ones72", [CO, 1], BF16, kind="ExternalInput").ap()
    o1x128 = nc.dram_tensor("o1x128", [1, 128], F32R, kind="ExternalInput").ap()
    ident = nc.dram_tensor("ident", [128, 128], BF16, kind="ExternalInput").ap()
    nrep = nc.dram_tensor("nrep", [1, 1], mybir.dt.int32, kind="ExternalInput").ap()
    oh = nc.dram_tensor("oh", [C, HALF, OW], F32, kind="ExternalOutput").ap()

    pool_taps = {ch: set(v) for ch, v in cfg["pool_taps"].items()}
    pe_leaves = tuple(cfg["pe_leaves"])
    merge_engines = cfg["merge_engines"]
    tree_taps = [t for t in range(9) if t not in pe_leaves]

    with tile.TileContext(nc) as tc, ExitStack() as ctx:
        cpool = ctx.enter_context(tc.tile_pool(name="consts", bufs=1))
        xpool = ctx.enter_context(tc.tile_pool(name="xpad", bufs=cfg["xpad_bufs"]))
        spool = ctx.enter_context(tc.tile_pool(name="small", bufs=3))
        sbcool = ctx.enter_context(tc.tile_pool(name="sbc", bufs=cfg["sbc_bufs"]))
        pkpool = ctx.enter_context(tc.tile_pool(name="pk", bufs=cfg["pk_bufs"]))
        accpool = ctx.enter_context(tc.tile_pool(name="acc", bufs=cfg["acc_bufs"]))
        outpool = ctx.enter_context(tc.tile_pool(name="outs", bufs=2))
        convp = ctx.enter_context(tc.tile_pool(name="convp", bufs=cfg["conv_bufs"], space="PSUM"))
        sgbp = ctx.enter_context(tc.tile_pool(name="sgbp", bufs=cfg["sigb_bufs"], space="PSUM"))
        denp = ctx.enter_context(tc.tile_pool(name="denp", bufs=1, space="PSUM"))
        rpsp = ctx.enter_context(tc.tile_pool(name="rpsp", bufs=1, space="PSUM"))
        saccp = ctx.enter_context(tc.tile_pool(name="saccp", bufs=cfg["sacc_bufs"], space="PSUM"))

        wsb = cpool.tile([128, 36 * CO], F32R, tag="wsb")
        SPLIT = 4 * CO
        nc.sync.dma_start(wsb[:, 0:SPLIT], wconv[:, 0:SPLIT])
        nc.sync.dma_start(wsb[:, SPLIT:], wconv[:, SPLIT:])
        selsb = cpool.tile([CO, 36 * 128], mybir.dt.float8e4, tag="selsb")
        nc.sync.dma_start(selsb[:], sel)
        bsb = cpool.tile([CO, 1], F32, tag="bsb")
        nc.sync.dma_start(bsb[:], bias)
        o72sb = cpool.tile([CO, 1], BF16, tag="o72sb")
        nc.sync.dma_start(o72sb[:], ones72)
        o1x128sb = cpool.tile([1, 128], F32R, tag="o1x128sb")
        nc.sync.dma_start(o1x128sb[:], o1x128)
        idsb = cpool.tile([128, 128], BF16, tag="idsb")
        nc.sync.dma_start(idsb[:], ident)
        rsb_t = cpool.tile([1, 1], mybir.dt.int32, tag="rsb_t")
        nc.sync.dma_start(rsb_t[:], nrep)
        if timing_loop:
            with tc.tile_critical():
                nrep_v = nc.values_load(rsb_t[:], min_val=1, max_val=1 << 20,
                                        skip_runtime_bounds_check=True)
            loop_cm = tc.For_i(0, nrep_v, 1, hint_engines=(
                mybir.EngineType.PE, mybir.EngineType.DVE,
                mybir.EngineType.Activation, mybir.EngineType.Pool,
                mybir.EngineType.SP))
        else:
            import contextlib
            loop_cm = contextlib.nullcontext()
        with loop_cm:
            for b in range(NBLK * unroll):
                b = b % NBLK
                xts = []
                _sec(f"xdma b{b}")
                import contextlib

                def prio(key):
                    v = cfg.get(key)
                    if v is None:
                        return contextlib.nullcontext()
                    return tc.high_priority(v if v > 0 else None)

                with prio("hp_xdma"):
                    for ch in range(2):
                        xt = xpool.tile([128, XROWS * XW], F32R, tag=f"x{ch}")
                        src = xh[128 * ch:128 * (ch + 1), 16 * b:16 * b + XROWS, :]
                        nc.sync.dma_start(
                            xt[:].rearrange("p (h w) -> p h w", w=XW), src)
                        xts.append(xt)

                def xview(t, ch):
                    ki, kj = divmod(t, 3)
                    v = xts[ch][:].rearrange("p (h w) -> p h w", w=XW)
                    return v[:, ki:ki + 2 * BROWS - 1:2, kj:kj + 2 * OW - 1:2]

                # ---- strided conv: 18 accumulating fp32r matmuls ----
                # row 72 of the tile doubles as the softmax-denominator row
                _sec(f"conv b{b}")
                cps_t = convp.tile([CO, BPIX], F32, tag="cps")
                cps = cps_t[:]
                for t in range(9):
                    for ch in range(2):
                        wv = wsb[:, (t * 2 + ch) * CO:(t * 2 + ch + 1) * CO]
                        nc.tensor.matmul(
                            cps.rearrange("p (h w) -> p h w", w=OW),
                            wv, xview(t, ch),
                            start=(t == 0 and ch == 0),
                            stop=(t == 8 and ch == 1))

                # ---- BN bias + exp (-> bf16) ----
                _sec(f"exp b{b}")
                with prio("hp_chain"):
                    sexp = spool.tile([CO, BPIX], BF16, tag="sexp")
                    nc.scalar.activation(sexp[:], cps[:], ACTF.Exp,
                                         bias=bsb[:], scale=1.0)

                    # ---- softmax denominator + reciprocal ----
                    _sec(f"den b{b}")
                    dps_t = denp.tile([1, BPIX], F32, tag="dps")
                    dps = dps_t[:]
                    nc.tensor.matmul(dps[:], o72sb[:], sexp[:],
                                     start=True, stop=True)
                    rrow = spool.tile([1, BPIX], F32, tag="rrow")
                    with nc.allow_low_precision(reason="18-bit recip, tol 2e-2"):
                        nc.vector.reciprocal_approx_fast(rrow[:], dps[:])
                    rrowr = spool.tile([1, BPIX], F32R, tag="rrowr")
                    nc.scalar.copy(rrowr[:], rrow[:])

                    # ---- broadcast recip to 128 rows, copy to SBUF bf16 ----
                    _sec(f"norm b{b}")
                    rps = rpsp.tile([128, BPIX], F32, tag="rps")
                    nc.tensor.matmul(rps[:], o1x128sb[:], rrowr[:],
                                     start=True, stop=True)
                    rB = spool.tile([128, BPIX], BF16, tag="rB")
                    nc.scalar.copy(rB[:], rps[:])
                sig = sexp

                # ---- per chunk: broadcast + weighted tap sum ----
                for ch in range(2):
                    unit = 2 * b + ch
                    pks = [None] * 9
                    for t in range(9):
                        _sec(f"tap{t} u{unit}")
                        sbp = sgbp.tile([128, BPIX], F32, tag="sbp")
                        sv = selsb[:, (t * 2 + ch) * 128:(t * 2 + ch + 1) * 128]
                        nc.tensor.matmul(sbp[:], sv, sig[:],
                                         start=True, stop=True)
                        pk = pkpool.tile([128, BPIX], BF16, tag="pk")
                        if t in pool_taps[ch]:
                            sbc = sbcool.tile([128, BPIX], BF16, tag="sbc")
                            nc.scalar.copy(sbc[:], sbp[:])
                            if cfg.get("pool_stt"):
                                nc.gpsimd.scalar_tensor_tensor(
                                    pk[:].rearrange("p (h w) -> p h w", w=OW),
                                    sbc[:].rearrange("p (h w) -> p h w", w=OW),
                                    1.0, xview(t, ch).bitcast(F32),
                                    ALU.bypass, ALU.mult)
                            else:
                                nc.gpsimd.tensor_tensor(
                                    pk[:].rearrange("p (h w) -> p h w", w=OW),
                                    sbc[:].rearrange("p (h w) -> p h w", w=OW),
                                    xview(t, ch).bitcast(F32), ALU.mult)
                        else:
                            nc.vector.scalar_tensor_tensor(
                                pk[:].rearrange("p (h w) -> p h w", w=OW),
                                sbp[:].rearrange("p (h w) -> p h w", w=OW),
                                1.0, xview(t, ch).bitcast(F32),
                                ALU.bypass, ALU.mult)
                        pks[t] = pk

                    # PE identity-matmul accumulation of pe_leaves
                    _sec(f"sacc u{unit}")
                    sacc = saccp.tile([128, BPIX], F32, tag="sacc")
                    for i, t in enumerate(pe_leaves):
                        nc.tensor.matmul(sacc[:], idsb[:], pks[t][:],
                                         start=(i == 0), stop=False)

                    # small add tree over the rest, injected into sacc by PE
                    _sec(f"tree u{unit}")
                    lvl = [pks[t] for t in tree_taps]
                    mi = 0
                    mengs = merge_engines[unit % len(merge_engines)]
                    while len(lvl) > 1:
                        nxt = []
                        for i in range(0, len(lvl) - 1, 2):
                            s = accpool.tile([128, BPIX], BF16, tag="acc")
                            eng = getattr(nc, mengs[mi % len(mengs)])
                            mi += 1
                            if eng is nc.gpsimd and cfg.get("pool_stt"):
                                eng.scalar_tensor_tensor(
                                    s[:], lvl[i][:], 1.0, lvl[i + 1][:],
                                    ALU.bypass, ALU.add)
                            else:
                                eng.tensor_add(s[:], lvl[i][:], lvl[i + 1][:])
                            nxt.append(s)
                        if len(lvl) % 2:
                            nxt.append(lvl[-1])
                        lvl = nxt
                    nc.tensor.matmul(sacc[:], idsb[:], lvl[0][:],
                                     start=False, stop=True)

                    # normalize from PSUM and write fp32 out
                    _sec(f"final u{unit}")
                    outsb = outpool.tile([128, BPIX], F32, tag="outsb")
                    nc.vector.scalar_tensor_tensor(
                        outsb[:], sacc[:], 1.0, rB[:], ALU.bypass, ALU.mult)
                    nc.sync.dma_start(
                        oh[128 * ch:128 * (ch + 1), BROWS * b:BROWS * (b + 1), :],
                        outsb[:].rearrange("p (h w) -> p h w", w=OW))
    nc.compile()
    return nc


def prepare_const_inputs(conv_w, bn_gamma, bn_beta, bn_mean, bn_var):
    inv = 1.0 / np.sqrt(bn_var.astype(np.float64) + BN_EPS)
    scale = (bn_gamma.astype(np.float64) * inv)
    wp = conv_w.astype(np.float64) * scale[:, None, None, None]
    bias = (bn_beta.astype(np.float64)
            - bn_mean.astype(np.float64) * scale).astype(np.float32)

    wconv = np.zeros((128, 36, CO), np.float32)
    selm = np.zeros((CO, 36, 128), np.float32)
    for t in range(9):
        ki, kj = divmod(t, 3)
        for ch in range(2):
            blk = t * 2 + ch
            wconv[:, blk, :] = wp[:, 128 * ch:128 * (ch + 1), ki, kj].T
            for g4 in range(4):
                r = (ch * 4 + g4) * 9 + t
                selm[r, blk, 32 * g4:32 * (g4 + 1)] = 1.0
    return {
        "wconv": np.ascontiguousarray(wconv.reshape(128, 36 * CO)),
        "sel": np.ascontiguousarray(
            selm.reshape(CO, 36 * 128)).astype(ml_dtypes.float8_e4m3),
        "bias": bias.reshape(CO, 1),
        "ones72": np.ones((CO, 1), ml_dtypes.bfloat16),
        "o1x128": np.ones((1, 128), np.float32),
        "ident": np.eye(128, dtype=ml_dtypes.bfloat16),
    }


def prepare_x_core(x, core):
    n, half = divmod(core, 2)
    if half == 0:
        rows = np.concatenate([x[n, :, 1:2, :], x[n, :, 0:2 * HALF, :]], axis=1)
    else:
        rows = x[n, :, 2 * HALF - 1:H, :]
    padded = np.concatenate([rows[:, :, 1:2], rows], axis=2)
    return np.ascontiguousarray(padded.astype(np.float32))


_CACHE = {}


def _get_nc(key="v5", cfg=None, timing_loop=True):
    if key not in _CACHE:
        nc = bacc.Bacc("TRN2", target_bir_lowering=False, debug=False,
                       num_devices=N_CORES)
        _CACHE[key] = build_kernel(nc, cfg=cfg or CFG, timing_loop=timing_loop)
    return _CACHE[key]


def run_on_cores(inputs, reps=1):
    nc = _get_nc()
    consts = prepare_const_inputs(
        inputs["conv_w"], inputs["bn_gamma"], inputs["bn_beta"],
        inputs["bn_mean"], inputs["bn_var"])
    consts["nrep"] = np.array([[reps]], np.int32)
    x = np.asarray(inputs["x"])
    in_maps = []
    for core in range(N_CORES):
        m = dict(consts)
        m["xh"] = prepare_x_core(x, core)
        in_maps.append(m)
    res = bass_utils.run_bass_kernel_spmd(nc, in_maps, core_ids=list(range(N_CORES)))
    out = np.empty((N, C, OH, OW), np.float32)
    for core in range(N_CORES):
        n, half = divmod(core, 2)
        out[n, :, HALF * half:HALF * (half + 1), :] = res.results[core]["oh"]
    return out


def kernel(**inputs):
    return run_on_cores(inputs, reps=1)
